# revision 17
# baseline (speedup 1.0000x reference)
"""Causal attention (naive double-normalize reference == causal softmax) on 8 TRN2 cores.

Key rewrite vs the AllGather-K/V design: scores = (x Wq)(x Wk)^T = (x Wqk) x^T
with Wqk = Wq Wk^T folded on the host. The score phase then contracts Q~ = x_q Wqk
against host-prepped x^T tiles streamed straight from HBM — no K projection and
no K AllGather at all. Only V still needs a collective (V = x_kv Wv is computed
sharded and AllGathered); it fires right after the V projection and its mesh
runs under the Q~ projection, well before AV consumes it.

Sharding:
  - Q rows interleaved: core i owns global rows {8l+i} -> uniform causal work per core.
  - V rows contiguous: core i computes rows [512i, 512(i+1)), AllGathers to all cores.

Per-core pipeline (all matmuls bf16 with fp32 PSUM accumulation):
  1. V  = x_kv Wv   [n_local, d] -> bounce -> AG_V (fires ~90us; mesh overlaps
     the Q~ projection, which needs no DMA by then).
  2. Q~T = Wqk^T x_q^T  [d, m_local].
  3. scores^T tiles ST[n_tile, m] = xT_tile^T . Q~T; exp(scale*s) -> P (bf16),
     causal mask on the 16-column diagonal straddle. xT tiles are kernel inputs:
     dep-free prefetch from t=0 on the gpsimd DMA ring.
  4. rowsum via 32 deferred ones-matmuls -> transpose (K=1 matmul) -> reciprocal.
  5. out[m, c] = sum_n P^T[n, m] V[n, c], scaled by reciprocal rowsum.

DMA ring split (per-engine queues dispatch in order and block at the head):
  - sync: xkv, wv, xq, wqk loads, then v_ag loads (head-blocked on AG_V done).
  - gpsimd: first 7 xT tiles, AG_V trigger, remaining xT tiles.
  - scalar: mask, v_bounce stores, out stores (ACT also does exp + out scaling).
"""

import math

import numpy as np

D = 2048          # d_in == d_out
CC = D // 128     # contraction chunks (16)
DT = D // 128     # output d tiles (16)
N_CORES = 8

_BUILT = {}


def _build(S):
    import concourse.bacc as bacc
    import concourse.mybir as mybir
    import concourse.tile as tile

    f32 = mybir.dt.float32
    bf16 = mybir.dt.bfloat16
    ML = S // N_CORES          # local q rows per core (512)
    NH = ML // 128             # output row tiles per core (4)
    NJ = S // 128              # key tiles (32)
    SCALE = 1.0 / math.sqrt(D)
    EXP = mybir.ActivationFunctionType.Exp
    CPY = mybir.ActivationFunctionType.Copy
    RG = [list(range(N_CORES))]
    XT_BUFS = 8                # rotating xT tiles (2 key tiles each)
    XT_TAIL = 4                # dedicated buffers for the last 4 xT tiles

    nc = bacc.Bacc("TRN2", target_bir_lowering=False)

    xq = nc.declare_dram_parameter("xq", [128, CC, ML], bf16, isOutput=False)
    xkv = nc.declare_dram_parameter("xkv", [128, CC, ML], bf16, isOutput=False)
    xt = nc.declare_dram_parameter("xt", [128, CC, S], bf16, isOutput=False)
    wqk = nc.declare_dram_parameter("wqk", [DT, 128, CC, 128], bf16, isOutput=False)
    wv = nc.declare_dram_parameter("wv", [128, CC, D], bf16, isOutput=False)
    maskp = nc.declare_dram_parameter("mask", [128, 16], bf16, isOutput=False)
    out = nc.declare_dram_parameter("out", [ML, D], f32, isOutput=True)

    with tile.TileContext(nc) as tc:
        with (
            tc.tile_pool(name="const", bufs=1) as const,
            tc.tile_pool(name="dram", bufs=1, space="DRAM") as dram,
            tc.tile_pool(name="xtstream", bufs=XT_BUFS) as xtstream,
        ):
            qt_sb = const.tile([128, CC, ML], bf16)
            mask_sb = const.tile([128, 16], bf16)
            ones_sb = const.tile([128, 1], bf16)
            one1_sb = const.tile([1, 1], f32)
            rs_sb = const.tile([1, ML], f32)
            rin_sb = const.tile([128, NH], f32)
            recip_sb = const.tile([128, NH], f32)

            warm_sb = const.tile([1, 1], f32)
            nc.vector.memset(ones_sb[:], 1.0)
            nc.vector.memset(one1_sb[:], 1.0)
            # Load the Exp activation table while the PE warms up, instead of
            # lazily on the first score tile (critical path).
            nc.scalar.activation(
                out=warm_sb[:], in_=one1_sb[:],
                func=mybir.ActivationFunctionType.Exp,
            )
            nc.scalar.dma_start(out=mask_sb[:], in_=maskp[:])

            v_bounce = dram.tile([ML, D], bf16)
            v_ag = dram.tile([S, D], bf16, addr_space="Shared")

            # xT prefetch: dep-free. Only 2 tiles fly before the V-proj input
            # loads (so they don't crowd xkv/wv off the wires); the rest queue
            # on the gpsimd ring behind the AG_V trigger, dispatching once the
            # V bounce is complete (~135us) — still well ahead of the score
            # phase consuming them.
            xt_tiles = {}

            def load_xt(tdx, eng):
                xtp = xtstream.tile([128, CC, 256], bf16, tag="xt", name=f"xt{tdx}")
                eng.dma_start(out=xtp[:], in_=xt[:, :, 256 * tdx:256 * (tdx + 1)])
                xt_tiles[tdx] = xtp

            for tdx in range(2):
                load_xt(tdx, nc.gpsimd)

            # ============ projections ============
            with (
                tc.tile_pool(name="px", bufs=1) as px,
                tc.tile_pool(name="stage", bufs=4) as stage,
                tc.tile_pool(name="proj_ps", bufs=4, space="PSUM") as proj_ps,
                tc.tile_pool(name="wvhold", bufs=4) as wvhold,
                tc.tile_pool(name="wqstream", bufs=12) as wqstream,
            ):
                xkv_sb = px.tile([128, CC, ML], bf16)
                xq_sb = px.tile([128, CC, ML], bf16)

                def load_wv(wc, wv_tiles):
                    wvt = wvhold.tile(
                        [128, CC, 256], bf16, tag="wv", name=f"wv{wc}"
                    )
                    nc.sync.dma_start(
                        out=wvt[:], in_=wv[:, :, 256 * wc:256 * (wc + 1)]
                    )
                    wv_tiles.append(wvt)

                # First-MM critical path: xkv and wv0/wv1 go first; later wv
                # tiles load in-loop (bufs=3) so the sync ring stays mostly
                # free for the xq/wqk prefetch that follows the loop.
                nc.sync.dma_start(out=xkv_sb[:, 0:4, :], in_=xkv[:, 0:4, :])
                nc.sync.dma_start(out=xkv_sb[:, 4:CC, :], in_=xkv[:, 4:CC, :])
                wv_tiles = []
                load_wv(0, wv_tiles)
                load_wv(1, wv_tiles)

                # ---- V projection -> bounce -> AG_V ----
                with tc.spectator_scope("vproj"):
                    for cs in range(4):
                        for half in range(2):
                            wc = 2 * cs + half
                            if wc >= 2:
                                load_wv(wc, wv_tiles)
                        for nt in range(ML // 128):
                            st = stage.tile(
                                [128, ML], bf16, tag="stage",
                                name=f"vst{cs}_{nt}",
                            )
                            for half in range(2):
                                wvt = wv_tiles[2 * cs + half]
                                ps = proj_ps.tile(
                                    [128, ML], f32, tag="proj",
                                    name=f"vps{cs}_{nt}_{half}",
                                )
                                for c in range(CC):
                                    nc.tensor.matmul(
                                        out=ps[:, 0:256],
                                        lhsT=xkv_sb[:, c, 128 * nt:128 * (nt + 1)],
                                        rhs=wvt[:, c, :],
                                        start=(c == 0), stop=(c == CC - 1),
                                    )
                                nc.vector.tensor_copy(
                                    out=st[:, 256 * half:256 * (half + 1)],
                                    in_=ps[:, 0:256],
                                )
                            nc.scalar.dma_start(
                                out=v_bounce[128 * nt:128 * (nt + 1),
                                             512 * cs:512 * (cs + 1)],
                                in_=st[:],
                            )
                    nc.gpsimd.collective_compute(
                        "AllGather", mybir.AluOpType.bypass,
                        replica_groups=RG,
                        ins=[v_bounce[:].opt()], outs=[v_ag[:].opt()],
                    )

                # ---- Q~ projection (wqk/xq prefetched on sync ring) ----
                nc.sync.dma_start(out=xq_sb[:], in_=xq[:])
                wq_tiles = []
                for dt in range(DT):
                    wqt = wqstream.tile(
                        [128, CC, 128], bf16, tag="wq", name=f"wq{dt}"
                    )
                    nc.sync.dma_start(out=wqt[:], in_=wqk[dt])
                    wq_tiles.append(wqt)
                # Bulk xT prefetch: on the sync ring AFTER the wqk loads — the
                # collective trigger does NOT gate the gpsimd ring, so putting
                # these there floods the wires at t=0 and starves xkv/wv.
                for tdx in range(2, XT_BUFS):
                    load_xt(tdx, nc.sync)
                with tc.spectator_scope("qtproj"):
                    for dt in range(DT):
                        ps = proj_ps.tile(
                            [128, ML], f32, tag="proj", name=f"qps{dt}"
                        )
                        for c in range(CC):
                            nc.tensor.matmul(
                                out=ps[:], lhsT=wq_tiles[dt][:, c, :],
                                rhs=xq_sb[:, c, :],
                                start=(c == 0), stop=(c == CC - 1),
                            )
                        nc.vector.tensor_copy(out=qt_sb[:, dt, :], in_=ps[:])

            # ============ attention ============
            with (
                tc.tile_pool(name="attn", bufs=1) as attn,
                tc.tile_pool(name="xttail", bufs=XT_TAIL) as xttail,
                tc.tile_pool(name="vstream", bufs=6) as vstream,
                tc.tile_pool(name="avstage", bufs=8) as avstage,
                tc.tile_pool(name="outp", bufs=4) as outp,
                tc.tile_pool(name="st_ps", bufs=2, space="PSUM") as st_ps,
                tc.tile_pool(name="rs_ps", bufs=1, space="PSUM") as rs_ps,
                tc.tile_pool(name="av_ps", bufs=1, space="PSUM") as av_ps,
                tc.tile_pool(name="tp_ps", bufs=1, space="PSUM") as tp_ps,
            ):
                p_all = attn.tile([128, NJ, ML], bf16)
                # Last 4 xT tiles go to dedicated buffers, loaded on the sync
                # ring AHEAD of the v_ag loads (emitted later) — otherwise the
                # v_ag prefetch flood starves the score tail's xT stream.
                for tdx in range(NJ // 2 - XT_TAIL, NJ // 2):
                    xtp = xttail.tile([128, CC, 256], bf16, tag="xtt", name=f"xtt{tdx}")
                    nc.sync.dma_start(out=xtp[:], in_=xt[:, :, 256 * tdx:256 * (tdx + 1)])
                    xt_tiles[tdx] = xtp
                with tc.spectator_scope("scores"):
                    for j in range(NJ):
                        tdx = j // 2
                        if j % 2 == 0:
                            if tdx not in xt_tiles:
                                load_xt(tdx, nc.gpsimd)
                            kt = xt_tiles[tdx][:, :, 0:128]
                        else:
                            kt = xt_tiles[tdx][:, :, 128:256]
                        m0 = 16 * j
                        ps = st_ps.tile([128, ML], f32, tag="st")
                        for c in range(CC):
                            nc.tensor.matmul(
                                out=ps[:, m0:ML], lhsT=kt[:, c, :],
                                rhs=qt_sb[:, c, m0:ML],
                                start=(c == 0), stop=(c == CC - 1),
                            )
                        pj = p_all[:, j, :]
                        nc.scalar.activation(
                            out=pj[:, m0:ML], in_=ps[:, m0:ML], func=EXP,
                            scale=SCALE,
                        )
                        nc.vector.tensor_tensor(
                            out=pj[:, m0:m0 + 16], in0=pj[:, m0:m0 + 16],
                            in1=mask_sb[:], op=mybir.AluOpType.mult,
                        )
                        g0 = 128 * (j // 8)
                        if m0 > g0:
                            nc.vector.memset(pj[:, g0:m0], 0.0)

                with tc.spectator_scope("renorm"):
                    # Deferred rowsum: all P tiles are ready, so these 32
                    # accumulating matmuls run back-to-back with no ACT/DVE
                    # round-trip in the PE queue.
                    rs = rs_ps.tile([1, ML], f32)
                    for j in range(NJ):
                        m0 = 16 * j
                        nc.tensor.matmul(
                            out=rs[0:1, m0:ML], lhsT=ones_sb[:],
                            rhs=p_all[:, j, m0:ML],
                            start=(j == 0), stop=(j == NJ - 1),
                        )
                    nc.vector.tensor_copy(out=rs_sb[:], in_=rs[:])
                    for h in range(NH):
                        tp = tp_ps.tile([128, 1], f32, tag="tp")
                        nc.tensor.matmul(
                            out=tp[:], lhsT=rs_sb[0:1, 128 * h:128 * (h + 1)],
                            rhs=one1_sb[:], start=True, stop=True,
                        )
                        nc.vector.tensor_copy(out=rin_sb[:, h:h + 1], in_=tp[:])
                    nc.vector.reciprocal(out=recip_sb[:], in_=rin_sb[:])

                with tc.spectator_scope("av"):
                    for cs in range(4):
                        av = [
                            av_ps.tile([128, 512], f32, tag=f"av{h}", name=f"av{h}_{cs}")
                            for h in range(NH)
                        ]
                        for t in range((NJ + 3) // 4):
                            vt = vstream.tile([128, 4, 512], bf16, tag="v")
                            nc.sync.dma_start(
                                out=vt[:],
                                in_=v_ag[512 * t:512 * (t + 1), 512 * cs:512 * (cs + 1)]
                                .rearrange("(jj p) n -> p jj n", p=128),
                            )
                            for jj in range(4):
                                j = 4 * t + jj
                                for h in range(j // 8, NH):
                                    nc.tensor.matmul(
                                        out=av[h][:],
                                        lhsT=p_all[:, j, 128 * h:128 * (h + 1)],
                                        rhs=vt[:, jj, :],
                                        start=(j == 0),
                                        stop=(j == min(8 * (h + 1), NJ) - 1),
                                    )
                            # Drain av[h] as soon as its accumulation stops
                            # (h's last key tile is j=8h+7, i.e. t=2h+1), so
                            # only h=NH-1's store trails the final matmul.
                            if t % 2 == 1:
                                h = (t - 1) // 2
                                # Unscaled copy frees the PSUM bank immediately
                                # so the next cs's accumulation never waits on
                                # the reciprocal (which waits on the whole
                                # score phase).
                                stg = avstage.tile(
                                    [128, 512], f32, tag="avs", name=f"avs{h}_{cs}"
                                )
                                nc.vector.tensor_copy(out=stg[:], in_=av[h][:])
                                ob = outp.tile([128, 512], f32, tag="out")
                                nc.scalar.activation(
                                    out=ob[:], in_=stg[:], func=CPY,
                                    scale=recip_sb[:, h:h + 1],
                                )
                                nc.scalar.dma_start(
                                    out=out[128 * h:128 * (h + 1), 512 * cs:512 * (cs + 1)],
                                    in_=ob[:],
                                )

    nc.finalize()
    return nc


def _prep_inputs(x, Wq, Wk, Wv, S):
    import ml_dtypes

    bf = ml_dtypes.bfloat16
    ML = S // N_CORES

    def shuf_w(W):
        # [dt, p, c, j] layout: element = W[128c+p, 128dt+j]
        return np.ascontiguousarray(
            W.reshape(CC, 128, DT, 128).transpose(2, 1, 0, 3)
        ).astype(bf)

    wqk_h = shuf_w((Wq @ Wk.T).astype(np.float32))
    wv_h = np.ascontiguousarray(
        Wv.reshape(CC, 128, D).transpose(1, 0, 2)
    ).astype(bf)

    def shuf_x(rows):
        # rows [n, D] -> [p, c, m] with element = rows[m, 128c+p]
        n = rows.shape[0]
        return np.ascontiguousarray(rows.reshape(n, CC, 128).transpose(2, 1, 0)).astype(bf)

    xt_h = shuf_x(x)
    in_maps = []
    for i in range(N_CORES):
        mask = (np.arange(128)[:, None] <= 8 * np.arange(16)[None, :] + i).astype(bf)
        in_maps.append({
            "xq": shuf_x(x[i::N_CORES]),
            "xkv": shuf_x(x[ML * i:ML * (i + 1)]),
            "xt": xt_h,
            "wqk": wqk_h, "wv": wv_h,
            "mask": mask,
        })
    return in_maps


def run(x, Wq, Wk, Wv, S, trace=False, trace_cores=None):
    from concourse.bass_utils import run_bass_kernel_spmd

    if S not in _BUILT:
        _BUILT[S] = _build(S)
    nc = _BUILT[S]
    in_maps = _prep_inputs(x, Wq, Wk, Wv, S)
    res = run_bass_kernel_spmd(
        nc, in_maps, list(range(N_CORES)), trace=trace, trace_cores=trace_cores
    )
    outs = [res.results[i]["out"] for i in range(N_CORES)]
    full = np.stack(outs, axis=1).reshape(S, D).astype(np.float32)
    return full, res


def kernel(x, Wq, Wk, Wv):
    x = np.asarray(x, dtype=np.float32)
    Wq = np.asarray(Wq, dtype=np.float32)
    Wk = np.asarray(Wk, dtype=np.float32)
    Wv = np.asarray(Wv, dtype=np.float32)
    full, _ = run(x, Wq, Wk, Wv, x.shape[0])
    return full


# revision 21
# speedup vs baseline: 1.0612x; 1.0612x over previous
"""Causal attention (naive double-normalize reference == causal softmax) on 8 TRN2 cores.

Key rewrite vs the AllGather-K/V design: scores = (x Wq)(x Wk)^T = (x Wqk) x^T
with Wqk = Wq Wk^T folded on the host. The score phase then contracts Q~ = x_q Wqk
against host-prepped x^T tiles streamed straight from HBM — no K projection and
no K AllGather at all. Only V still needs a collective (V = x_kv Wv is computed
sharded and AllGathered); it fires right after the V projection and its mesh
runs under the Q~ projection, well before AV consumes it.

Sharding:
  - Q rows interleaved: core i owns global rows {8l+i} -> uniform causal work per core.
  - V rows contiguous: core i computes rows [512i, 512(i+1)), AllGathers to all cores.

Per-core pipeline (all matmuls bf16 with fp32 PSUM accumulation):
  1. V  = x_kv Wv   [n_local, d] -> bounce -> AG_V (fires ~90us; mesh overlaps
     the Q~ projection, which needs no DMA by then).
  2. Q~T = Wqk^T x_q^T  [d, m_local].
  3. scores^T tiles ST[n_tile, m] = xT_tile^T . Q~T; exp(scale*s) -> P (bf16),
     causal mask on the 16-column diagonal straddle. xT tiles are kernel inputs:
     dep-free prefetch from t=0 on the gpsimd DMA ring.
  4. rowsum via 32 deferred ones-matmuls -> transpose (K=1 matmul) -> reciprocal.
  5. out[m, c] = sum_n P^T[n, m] V[n, c], scaled by reciprocal rowsum.

DMA ring split (per-engine queues dispatch in order and block at the head):
  - sync: xkv, wv, xq, wqk loads, then v_ag loads (head-blocked on AG_V done).
  - gpsimd: first 7 xT tiles, AG_V trigger, remaining xT tiles.
  - scalar: mask, v_bounce stores, out stores (ACT also does exp + out scaling).
"""

import math

import numpy as np

D = 2048          # d_in == d_out
CC = D // 128     # contraction chunks (16)
DT = D // 128     # output d tiles (16)
N_CORES = 8

_BUILT = {}


def _build(S):
    import concourse.bacc as bacc
    import concourse.mybir as mybir
    import concourse.tile as tile

    f32 = mybir.dt.float32
    bf16 = mybir.dt.bfloat16
    ML = S // N_CORES          # local q rows per core (512)
    NH = ML // 128             # output row tiles per core (4)
    NJ = S // 128              # key tiles (32)
    SCALE = 1.0 / math.sqrt(D)
    EXP = mybir.ActivationFunctionType.Exp
    CPY = mybir.ActivationFunctionType.Copy
    RG = [list(range(N_CORES))]
    XT_BUFS = 9                # rotating xT tiles (2 key tiles each)

    nc = bacc.Bacc("TRN2", target_bir_lowering=False)

    xq = nc.declare_dram_parameter("xq", [128, CC, ML], bf16, isOutput=False)
    xkv = nc.declare_dram_parameter("xkv", [128, CC, ML], bf16, isOutput=False)
    xt = nc.declare_dram_parameter("xt", [128, CC, S], bf16, isOutput=False)
    wqk = nc.declare_dram_parameter("wqk", [DT, 128, CC, 128], bf16, isOutput=False)
    wv = nc.declare_dram_parameter("wv", [128, CC, D], bf16, isOutput=False)
    maskp = nc.declare_dram_parameter("mask", [128, 16], bf16, isOutput=False)
    out = nc.declare_dram_parameter("out", [ML, D], f32, isOutput=True)

    with tile.TileContext(nc) as tc:
        with (
            tc.tile_pool(name="const", bufs=1) as const,
            tc.tile_pool(name="dram", bufs=1, space="DRAM") as dram,
            tc.tile_pool(name="xtstream", bufs=XT_BUFS) as xtstream,
        ):
            qt_sb = const.tile([128, CC, ML], bf16)
            mask_sb = const.tile([128, 16], bf16)
            ones_sb = const.tile([128, 1], bf16)
            one1_sb = const.tile([1, 1], f32)
            rs_sb = const.tile([1, ML], f32)
            rin_sb = const.tile([128, NH], f32)
            recip_sb = const.tile([128, NH], f32)

            warm_sb = const.tile([1, 1], f32)
            nc.vector.memset(ones_sb[:], 1.0)
            nc.vector.memset(one1_sb[:], 1.0)
            # Load the Exp activation table while the PE warms up, instead of
            # lazily on the first score tile (critical path).
            nc.scalar.activation(
                out=warm_sb[:], in_=one1_sb[:],
                func=mybir.ActivationFunctionType.Exp,
            )
            nc.scalar.dma_start(out=mask_sb[:], in_=maskp[:])

            v_bounce = dram.tile([ML, D], bf16)
            v_ag = dram.tile([S, D], bf16, addr_space="Shared")

            # xT prefetch: dep-free. Only 2 tiles fly before the V-proj input
            # loads (so they don't crowd xkv/wv off the wires); the rest queue
            # on the gpsimd ring behind the AG_V trigger, dispatching once the
            # V bounce is complete (~135us) — still well ahead of the score
            # phase consuming them.
            xt_tiles = {}

            def load_xt(tdx, eng):
                xtp = xtstream.tile([128, CC, 256], bf16, tag="xt", name=f"xt{tdx}")
                eng.dma_start(out=xtp[:], in_=xt[:, :, 256 * tdx:256 * (tdx + 1)])
                xt_tiles[tdx] = xtp

            for tdx in range(2):
                load_xt(tdx, nc.gpsimd)

            # ============ projections ============
            with (
                tc.tile_pool(name="px", bufs=1) as px,
                tc.tile_pool(name="stage", bufs=4) as stage,
                tc.tile_pool(name="proj_ps", bufs=4, space="PSUM") as proj_ps,
                tc.tile_pool(name="wvhold", bufs=3) as wvhold,
                tc.tile_pool(name="wqstream", bufs=12) as wqstream,
            ):
                xkv_sb = px.tile([128, CC, ML], bf16)
                xq_sb = px.tile([128, CC, ML], bf16)

                def load_wv(wc, wv_tiles):
                    wvt = wvhold.tile(
                        [128, CC, 256], bf16, tag="wv", name=f"wv{wc}"
                    )
                    nc.sync.dma_start(
                        out=wvt[:], in_=wv[:, :, 256 * wc:256 * (wc + 1)]
                    )
                    wv_tiles.append(wvt)

                # First-MM critical path: xkv and wv0/wv1 go first; later wv
                # tiles load in-loop (bufs=3) so the sync ring stays mostly
                # free for the xq/wqk prefetch that follows the loop.
                nc.sync.dma_start(out=xkv_sb[:, 0:4, :], in_=xkv[:, 0:4, :])
                nc.sync.dma_start(out=xkv_sb[:, 4:CC, :], in_=xkv[:, 4:CC, :])
                wv_tiles = []
                load_wv(0, wv_tiles)
                load_wv(1, wv_tiles)

                # ---- V projection -> bounce -> AG_V ----
                with tc.spectator_scope("vproj"):
                    for cs in range(4):
                        for half in range(2):
                            wc = 2 * cs + half
                            if wc >= 2:
                                load_wv(wc, wv_tiles)
                        for nt in range(ML // 128):
                            st = stage.tile(
                                [128, ML], bf16, tag="stage",
                                name=f"vst{cs}_{nt}",
                            )
                            for half in range(2):
                                wvt = wv_tiles[2 * cs + half]
                                ps = proj_ps.tile(
                                    [128, ML], f32, tag="proj",
                                    name=f"vps{cs}_{nt}_{half}",
                                )
                                for c in range(CC):
                                    nc.tensor.matmul(
                                        out=ps[:, 0:256],
                                        lhsT=xkv_sb[:, c, 128 * nt:128 * (nt + 1)],
                                        rhs=wvt[:, c, :],
                                        start=(c == 0), stop=(c == CC - 1),
                                    )
                                nc.vector.tensor_copy(
                                    out=st[:, 256 * half:256 * (half + 1)],
                                    in_=ps[:, 0:256],
                                )
                            nc.scalar.dma_start(
                                out=v_bounce[128 * nt:128 * (nt + 1),
                                             512 * cs:512 * (cs + 1)],
                                in_=st[:],
                            )
                    nc.gpsimd.collective_compute(
                        "AllGather", mybir.AluOpType.bypass,
                        replica_groups=RG,
                        ins=[v_bounce[:].opt()], outs=[v_ag[:].opt()],
                    )

                # ---- Q~ projection (wqk/xq prefetched on sync ring) ----
                nc.sync.dma_start(out=xq_sb[:], in_=xq[:])
                wq_tiles = []
                for dt in range(DT):
                    wqt = wqstream.tile(
                        [128, CC, 128], bf16, tag="wq", name=f"wq{dt}"
                    )
                    nc.sync.dma_start(out=wqt[:], in_=wqk[dt])
                    wq_tiles.append(wqt)
                # Bulk xT prefetch: on the sync ring AFTER the wqk loads — the
                # collective trigger does NOT gate the gpsimd ring, so putting
                # these there floods the wires at t=0 and starves xkv/wv.
                for tdx in range(2, XT_BUFS):
                    load_xt(tdx, nc.sync)
                with tc.spectator_scope("qtproj"):
                    for dt in range(DT):
                        ps = proj_ps.tile(
                            [128, ML], f32, tag="proj", name=f"qps{dt}"
                        )
                        for c in range(CC):
                            nc.tensor.matmul(
                                out=ps[:], lhsT=wq_tiles[dt][:, c, :],
                                rhs=xq_sb[:, c, :],
                                start=(c == 0), stop=(c == CC - 1),
                            )
                        nc.vector.tensor_copy(out=qt_sb[:, dt, :], in_=ps[:])

            # ============ attention ============
            with (
                tc.tile_pool(name="attn", bufs=1) as attn,
                tc.tile_pool(name="vstream", bufs=8) as vstream,
                tc.tile_pool(name="avstage", bufs=8) as avstage,
                tc.tile_pool(name="outp", bufs=4) as outp,
                tc.tile_pool(name="st_ps", bufs=2, space="PSUM") as st_ps,
                tc.tile_pool(name="rs_ps", bufs=1, space="PSUM") as rs_ps,
                tc.tile_pool(name="av_ps", bufs=1, space="PSUM") as av_ps,
                tc.tile_pool(name="tp_ps", bufs=1, space="PSUM") as tp_ps,
            ):
                p_all = attn.tile([128, NJ, ML], bf16)
                with tc.spectator_scope("scores"):
                    for j in range(NJ):
                        tdx = j // 2
                        if j % 2 == 0:
                            if tdx not in xt_tiles:
                                load_xt(tdx, nc.gpsimd)
                            kt = xt_tiles[tdx][:, :, 0:128]
                        else:
                            kt = xt_tiles[tdx][:, :, 128:256]
                        m0 = 16 * j
                        ps = st_ps.tile([128, ML], f32, tag="st")
                        for c in range(CC):
                            nc.tensor.matmul(
                                out=ps[:, m0:ML], lhsT=kt[:, c, :],
                                rhs=qt_sb[:, c, m0:ML],
                                start=(c == 0), stop=(c == CC - 1),
                            )
                        pj = p_all[:, j, :]
                        nc.scalar.activation(
                            out=pj[:, m0:ML], in_=ps[:, m0:ML], func=EXP,
                            scale=SCALE,
                        )
                        nc.vector.tensor_tensor(
                            out=pj[:, m0:m0 + 16], in0=pj[:, m0:m0 + 16],
                            in1=mask_sb[:], op=mybir.AluOpType.mult,
                        )
                        g0 = 128 * (j // 8)
                        if m0 > g0:
                            nc.vector.memset(pj[:, g0:m0], 0.0)

                with tc.spectator_scope("renorm"):
                    # Deferred rowsum: all P tiles are ready, so these 32
                    # accumulating matmuls run back-to-back with no ACT/DVE
                    # round-trip in the PE queue.
                    rs = rs_ps.tile([1, ML], f32)
                    for j in range(NJ):
                        m0 = 16 * j
                        nc.tensor.matmul(
                            out=rs[0:1, m0:ML], lhsT=ones_sb[:],
                            rhs=p_all[:, j, m0:ML],
                            start=(j == 0), stop=(j == NJ - 1),
                        )
                    nc.vector.tensor_copy(out=rs_sb[:], in_=rs[:])
                    for h in range(NH):
                        tp = tp_ps.tile([128, 1], f32, tag="tp")
                        nc.tensor.matmul(
                            out=tp[:], lhsT=rs_sb[0:1, 128 * h:128 * (h + 1)],
                            rhs=one1_sb[:], start=True, stop=True,
                        )
                        nc.vector.tensor_copy(out=rin_sb[:, h:h + 1], in_=tp[:])
                    nc.vector.reciprocal(out=recip_sb[:], in_=rin_sb[:])

                with tc.spectator_scope("av"):
                    for cs in range(4):
                        av = [
                            av_ps.tile([128, 512], f32, tag=f"av{h}", name=f"av{h}_{cs}")
                            for h in range(NH)
                        ]
                        for t in range((NJ + 3) // 4):
                            vt = vstream.tile([128, 4, 512], bf16, tag="v")
                            nc.sync.dma_start(
                                out=vt[:],
                                in_=v_ag[512 * t:512 * (t + 1), 512 * cs:512 * (cs + 1)]
                                .rearrange("(jj p) n -> p jj n", p=128),
                            )
                            for jj in range(4):
                                j = 4 * t + jj
                                for h in range(j // 8, NH):
                                    nc.tensor.matmul(
                                        out=av[h][:],
                                        lhsT=p_all[:, j, 128 * h:128 * (h + 1)],
                                        rhs=vt[:, jj, :],
                                        start=(j == 0),
                                        stop=(j == min(8 * (h + 1), NJ) - 1),
                                    )
                            # Drain av[h] as soon as its accumulation stops
                            # (h's last key tile is j=8h+7, i.e. t=2h+1), so
                            # only h=NH-1's store trails the final matmul.
                            if t % 2 == 1:
                                h = (t - 1) // 2
                                # Unscaled copy frees the PSUM bank immediately
                                # so the next cs's accumulation never waits on
                                # the reciprocal (which waits on the whole
                                # score phase).
                                stg = avstage.tile(
                                    [128, 512], f32, tag="avs", name=f"avs{h}_{cs}"
                                )
                                nc.vector.tensor_copy(out=stg[:], in_=av[h][:])
                                ob = outp.tile([128, 512], f32, tag="out")
                                nc.scalar.activation(
                                    out=ob[:], in_=stg[:], func=CPY,
                                    scale=recip_sb[:, h:h + 1],
                                )
                                nc.scalar.dma_start(
                                    out=out[128 * h:128 * (h + 1), 512 * cs:512 * (cs + 1)],
                                    in_=ob[:],
                                )

    nc.finalize()
    return nc


def _prep_inputs(x, Wq, Wk, Wv, S):
    import ml_dtypes

    bf = ml_dtypes.bfloat16
    ML = S // N_CORES

    def shuf_w(W):
        # [dt, p, c, j] layout: element = W[128c+p, 128dt+j]
        return np.ascontiguousarray(
            W.reshape(CC, 128, DT, 128).transpose(2, 1, 0, 3)
        ).astype(bf)

    wqk_h = shuf_w((Wq @ Wk.T).astype(np.float32))
    wv_h = np.ascontiguousarray(
        Wv.reshape(CC, 128, D).transpose(1, 0, 2)
    ).astype(bf)

    def shuf_x(rows):
        # rows [n, D] -> [p, c, m] with element = rows[m, 128c+p]
        n = rows.shape[0]
        return np.ascontiguousarray(rows.reshape(n, CC, 128).transpose(2, 1, 0)).astype(bf)

    xt_h = shuf_x(x)
    in_maps = []
    for i in range(N_CORES):
        mask = (np.arange(128)[:, None] <= 8 * np.arange(16)[None, :] + i).astype(bf)
        in_maps.append({
            "xq": shuf_x(x[i::N_CORES]),
            "xkv": shuf_x(x[ML * i:ML * (i + 1)]),
            "xt": xt_h,
            "wqk": wqk_h, "wv": wv_h,
            "mask": mask,
        })
    return in_maps


def run(x, Wq, Wk, Wv, S, trace=False, trace_cores=None):
    from concourse.bass_utils import run_bass_kernel_spmd

    if S not in _BUILT:
        _BUILT[S] = _build(S)
    nc = _BUILT[S]
    in_maps = _prep_inputs(x, Wq, Wk, Wv, S)
    res = run_bass_kernel_spmd(
        nc, in_maps, list(range(N_CORES)), trace=trace, trace_cores=trace_cores
    )
    outs = [res.results[i]["out"] for i in range(N_CORES)]
    full = np.stack(outs, axis=1).reshape(S, D).astype(np.float32)
    return full, res


def kernel(x, Wq, Wk, Wv):
    x = np.asarray(x, dtype=np.float32)
    Wq = np.asarray(Wq, dtype=np.float32)
    Wk = np.asarray(Wk, dtype=np.float32)
    Wv = np.asarray(Wv, dtype=np.float32)
    full, _ = run(x, Wq, Wk, Wv, x.shape[0])
    return full


# revision 30
# speedup vs baseline: 1.1286x; 1.0635x over previous
"""Causal attention (naive double-normalize reference == causal softmax) on 8 TRN2 cores.

Key rewrite vs the AllGather-K/V design: scores = (x Wq)(x Wk)^T = (x Wqk) x^T
with Wqk = Wq Wk^T folded on the host. The score phase then contracts Q~ = x_q Wqk
against host-prepped x^T tiles streamed straight from HBM — no K projection and
no K AllGather at all. Only V still needs a collective (V = x_kv Wv is computed
sharded and AllGathered); it fires right after the V projection and its mesh
runs under the Q~ projection, well before AV consumes it.

Sharding:
  - Q rows interleaved: core i owns global rows {8l+i} -> uniform causal work per core.
  - V rows contiguous: core i computes rows [512i, 512(i+1)), AllGathers to all cores.

Per-core pipeline (all matmuls bf16 with fp32 PSUM accumulation):
  1. V  = x_kv Wv   [n_local, d] -> bounce -> AG_V (fires ~90us; mesh overlaps
     the Q~ projection, which needs no DMA by then).
  2. Q~T = Wqk^T x_q^T  [d, m_local].
  3. scores^T tiles ST[n_tile, m] = xT_tile^T . Q~T; exp(scale*s) -> P (bf16),
     causal mask on the 16-column diagonal straddle. xT tiles are kernel inputs:
     dep-free prefetch from t=0 on the gpsimd DMA ring.
  4. rowsum via 32 deferred ones-matmuls -> transpose (K=1 matmul) -> reciprocal.
  5. out[m, c] = sum_n P^T[n, m] V[n, c], scaled by reciprocal rowsum.

DMA ring split (per-engine queues dispatch in order and block at the head):
  - sync: xkv, wv, xq, wqk loads, then v_ag loads (head-blocked on AG_V done).
  - gpsimd: first 7 xT tiles, AG_V trigger, remaining xT tiles.
  - scalar: mask, v_bounce stores, out stores (ACT also does exp + out scaling).
"""

import math

import numpy as np

D = 2048          # d_in == d_out
CC = D // 128     # contraction chunks (16)
DT = D // 128     # output d tiles (16)
N_CORES = 8

_BUILT = {}


def _build(S):
    import concourse.bacc as bacc
    import concourse.mybir as mybir
    import concourse.tile as tile

    f32 = mybir.dt.float32
    bf16 = mybir.dt.bfloat16
    ML = S // N_CORES          # local q rows per core (512)
    NH = ML // 128             # output row tiles per core (4)
    NJ = S // 128              # key tiles (32)
    SCALE = 1.0 / math.sqrt(D)
    EXP = mybir.ActivationFunctionType.Exp
    CPY = mybir.ActivationFunctionType.Copy
    RG = [list(range(N_CORES))]
    XT_BUFS = 8                # rotating xT tiles (2 key tiles each)
    XT_TAIL = 4                # dedicated buffers for the last 4 xT tiles

    nc = bacc.Bacc("TRN2", target_bir_lowering=False)

    xq = nc.declare_dram_parameter("xq", [128, CC, ML], bf16, isOutput=False)
    xkv = nc.declare_dram_parameter("xkv", [128, CC, ML], bf16, isOutput=False)
    xt = nc.declare_dram_parameter("xt", [128, CC, S], bf16, isOutput=False)
    wqk = nc.declare_dram_parameter("wqk", [DT, 128, CC, 128], bf16, isOutput=False)
    wv = nc.declare_dram_parameter("wv", [128, CC, D], bf16, isOutput=False)
    maskp = nc.declare_dram_parameter("mask", [128, 16], bf16, isOutput=False)
    out = nc.declare_dram_parameter("out", [ML, D], f32, isOutput=True)

    with tile.TileContext(nc) as tc:
        with (
            tc.tile_pool(name="const", bufs=1) as const,
            tc.tile_pool(name="dram", bufs=1, space="DRAM") as dram,
            tc.tile_pool(name="xtstream", bufs=XT_BUFS) as xtstream,
        ):
            qt_sb = const.tile([128, CC, ML], bf16)
            mask_sb = const.tile([128, 16], bf16)
            ones_sb = const.tile([128, 1], bf16)
            one1_sb = const.tile([1, 1], f32)
            rs_sb = const.tile([1, ML], f32)
            rin_sb = const.tile([128, NH], f32)
            recip_sb = const.tile([128, NH], f32)

            warm_sb = const.tile([1, 1], f32)
            nc.vector.memset(ones_sb[:], 1.0)
            nc.vector.memset(one1_sb[:], 1.0)
            # Load the Exp activation table while the PE warms up, instead of
            # lazily on the first score tile (critical path).
            nc.scalar.activation(
                out=warm_sb[:], in_=one1_sb[:],
                func=mybir.ActivationFunctionType.Exp,
            )
            nc.scalar.dma_start(out=mask_sb[:], in_=maskp[:])

            # V is gathered in two d-halves: AV's first two column blocks only
            # need V[:, 0:1024], so AG_V1 fires after half the V projection and
            # AV is no longer gated on the full gather completing (~270us).
            v_bounce = [dram.tile([ML, D // 2], bf16, name=f"vb{i}") for i in range(2)]
            v_ag = [
                dram.tile([S, D // 2], bf16, addr_space="Shared", name=f"vag{i}")
                for i in range(2)
            ]

            # xT prefetch: dep-free. Only 2 tiles fly before the V-proj input
            # loads (so they don't crowd xkv/wv off the wires); the rest queue
            # on the gpsimd ring behind the AG_V trigger, dispatching once the
            # V bounce is complete (~135us) — still well ahead of the score
            # phase consuming them.
            xt_tiles = {}

            def load_xt(tdx, eng):
                xtp = xtstream.tile([128, CC, 256], bf16, tag="xt", name=f"xt{tdx}")
                eng.dma_start(out=xtp[:], in_=xt[:, :, 256 * tdx:256 * (tdx + 1)])
                xt_tiles[tdx] = xtp



            # ============ projections ============
            with (
                tc.tile_pool(name="px", bufs=1) as px,
                tc.tile_pool(name="stage", bufs=4) as stage,
                tc.tile_pool(name="proj_ps", bufs=4, space="PSUM") as proj_ps,
                tc.tile_pool(name="wvhold", bufs=4) as wvhold,
                tc.tile_pool(name="wqstream", bufs=12) as wqstream,
            ):
                xkv_sb = px.tile([128, CC, ML], bf16)
                xq_sb = px.tile([128, CC, ML], bf16)

                def load_wv(wc, wv_tiles):
                    wvt = wvhold.tile(
                        [128, CC, 256], bf16, tag="wv", name=f"wv{wc}"
                    )
                    nc.sync.dma_start(
                        out=wvt[:], in_=wv[:, :, 256 * wc:256 * (wc + 1)]
                    )
                    wv_tiles.append(wvt)

                # First-MM critical path: xkv and wv0/wv1 go first; later wv
                # tiles load in-loop (bufs=3) so the sync ring stays mostly
                # free for the xq/wqk prefetch that follows the loop.
                nc.sync.dma_start(out=xkv_sb[:, 0:4, :], in_=xkv[:, 0:4, :])
                nc.sync.dma_start(out=xkv_sb[:, 4:CC, :], in_=xkv[:, 4:CC, :])
                wv_tiles = []
                load_wv(0, wv_tiles)
                load_wv(1, wv_tiles)

                # ---- V projection -> bounce -> AG_V ----
                with tc.spectator_scope("vproj"):
                    for cs in range(4):
                        for half in range(2):
                            wc = 2 * cs + half
                            if wc >= 2:
                                load_wv(wc, wv_tiles)
                        for nt in range(ML // 128):
                            st = stage.tile(
                                [128, ML], bf16, tag="stage",
                                name=f"vst{cs}_{nt}",
                            )
                            for half in range(2):
                                wvt = wv_tiles[2 * cs + half]
                                ps = proj_ps.tile(
                                    [128, ML], f32, tag="proj",
                                    name=f"vps{cs}_{nt}_{half}",
                                )
                                for c in range(CC):
                                    nc.tensor.matmul(
                                        out=ps[:, 0:256],
                                        lhsT=xkv_sb[:, c, 128 * nt:128 * (nt + 1)],
                                        rhs=wvt[:, c, :],
                                        start=(c == 0), stop=(c == CC - 1),
                                    )
                                nc.vector.tensor_copy(
                                    out=st[:, 256 * half:256 * (half + 1)],
                                    in_=ps[:, 0:256],
                                )
                            half_id, c0 = cs // 2, 512 * (cs % 2)
                            nc.scalar.dma_start(
                                out=v_bounce[half_id][128 * nt:128 * (nt + 1),
                                                      c0:c0 + 512],
                                in_=st[:],
                            )
                        if cs % 2 == 1:
                            nc.gpsimd.collective_compute(
                                "AllGather", mybir.AluOpType.bypass,
                                replica_groups=RG,
                                ins=[v_bounce[cs // 2][:].opt()],
                                outs=[v_ag[cs // 2][:].opt()],
                            )

                # ---- Q~ projection (wqk/xq prefetched on sync ring) ----
                nc.sync.dma_start(out=xq_sb[:], in_=xq[:])
                wq_tiles = []
                for dt in range(DT):
                    wqt = wqstream.tile(
                        [128, CC, 128], bf16, tag="wq", name=f"wq{dt}"
                    )
                    nc.sync.dma_start(out=wqt[:], in_=wqk[dt])
                    wq_tiles.append(wqt)
                # Bulk xT prefetch: on the sync ring AFTER the wqk loads — the
                # collective trigger does NOT gate the gpsimd ring, so putting
                # these there floods the wires at t=0 and starves xkv/wv.
                for tdx in range(XT_BUFS):
                    load_xt(tdx, nc.sync)
                with tc.spectator_scope("qtproj"):
                    for dt in range(DT):
                        ps = proj_ps.tile(
                            [128, ML], f32, tag="proj", name=f"qps{dt}"
                        )
                        for c in range(CC):
                            nc.tensor.matmul(
                                out=ps[:], lhsT=wq_tiles[dt][:, c, :],
                                rhs=xq_sb[:, c, :],
                                start=(c == 0), stop=(c == CC - 1),
                            )
                        nc.vector.tensor_copy(out=qt_sb[:, dt, :], in_=ps[:])

            # ============ attention ============
            with (
                tc.tile_pool(name="attn", bufs=1) as attn,
                tc.tile_pool(name="xttail", bufs=XT_TAIL) as xttail,
                tc.tile_pool(name="vstream", bufs=8) as vstream,
                tc.tile_pool(name="avstage", bufs=8) as avstage,
                tc.tile_pool(name="outp", bufs=4) as outp,
                tc.tile_pool(name="st_ps", bufs=2, space="PSUM") as st_ps,
                tc.tile_pool(name="rs_ps", bufs=1, space="PSUM") as rs_ps,
                tc.tile_pool(name="av_ps", bufs=1, space="PSUM") as av_ps,
                tc.tile_pool(name="tp_ps", bufs=1, space="PSUM") as tp_ps,
            ):
                p_all = attn.tile([128, NJ, ML], bf16)
                # Last 4 xT tiles in dedicated buffers, loaded on the sync ring
                # AHEAD of the v_ag loads — the v_ag prefetch flood otherwise
                # starves the score tail's xT stream.
                for tdx in range(NJ // 2 - XT_TAIL, NJ // 2):
                    xtp = xttail.tile([128, CC, 256], bf16, tag="xtt", name=f"xtt{tdx}")
                    nc.sync.dma_start(out=xtp[:], in_=xt[:, :, 256 * tdx:256 * (tdx + 1)])
                    xt_tiles[tdx] = xtp
                with tc.spectator_scope("scores"):
                    for j in range(NJ):
                        tdx = j // 2
                        if j % 2 == 0:
                            if tdx not in xt_tiles:
                                load_xt(tdx, nc.gpsimd)
                            kt = xt_tiles[tdx][:, :, 0:128]
                        else:
                            kt = xt_tiles[tdx][:, :, 128:256]
                        m0 = 16 * j
                        ps = st_ps.tile([128, ML], f32, tag="st")
                        for c in range(CC):
                            nc.tensor.matmul(
                                out=ps[:, m0:ML], lhsT=kt[:, c, :],
                                rhs=qt_sb[:, c, m0:ML],
                                start=(c == 0), stop=(c == CC - 1),
                            )
                        pj = p_all[:, j, :]
                        nc.scalar.activation(
                            out=pj[:, m0:ML], in_=ps[:, m0:ML], func=EXP,
                            scale=SCALE,
                        )
                        nc.vector.tensor_tensor(
                            out=pj[:, m0:m0 + 16], in0=pj[:, m0:m0 + 16],
                            in1=mask_sb[:], op=mybir.AluOpType.mult,
                        )
                        g0 = 128 * (j // 8)
                        if m0 > g0:
                            nc.vector.memset(pj[:, g0:m0], 0.0)

                with tc.spectator_scope("renorm"):
                    # Deferred rowsum: all P tiles are ready, so these 32
                    # accumulating matmuls run back-to-back with no ACT/DVE
                    # round-trip in the PE queue.
                    rs = rs_ps.tile([1, ML], f32)
                    for j in range(NJ):
                        m0 = 16 * j
                        nc.tensor.matmul(
                            out=rs[0:1, m0:ML], lhsT=ones_sb[:],
                            rhs=p_all[:, j, m0:ML],
                            start=(j == 0), stop=(j == NJ - 1),
                        )
                    nc.vector.tensor_copy(out=rs_sb[:], in_=rs[:])
                    for h in range(NH):
                        tp = tp_ps.tile([128, 1], f32, tag="tp")
                        nc.tensor.matmul(
                            out=tp[:], lhsT=rs_sb[0:1, 128 * h:128 * (h + 1)],
                            rhs=one1_sb[:], start=True, stop=True,
                        )
                        nc.vector.tensor_copy(out=rin_sb[:, h:h + 1], in_=tp[:])
                    nc.vector.reciprocal(out=recip_sb[:], in_=rin_sb[:])

                with tc.spectator_scope("av"):
                    for cs in range(4):
                        av = [
                            av_ps.tile([128, 512], f32, tag=f"av{h}", name=f"av{h}_{cs}")
                            for h in range(NH)
                        ]
                        for t in range((NJ + 3) // 4):
                            vt = vstream.tile([128, 4, 512], bf16, tag="v")
                            c0 = 512 * (cs % 2)
                            nc.sync.dma_start(
                                out=vt[:],
                                in_=v_ag[cs // 2][512 * t:512 * (t + 1), c0:c0 + 512]
                                .rearrange("(jj p) n -> p jj n", p=128),
                            )
                            for jj in range(4):
                                j = 4 * t + jj
                                for h in range(j // 8, NH):
                                    nc.tensor.matmul(
                                        out=av[h][:],
                                        lhsT=p_all[:, j, 128 * h:128 * (h + 1)],
                                        rhs=vt[:, jj, :],
                                        start=(j == 0),
                                        stop=(j == min(8 * (h + 1), NJ) - 1),
                                    )
                            # Drain av[h] as soon as its accumulation stops
                            # (h's last key tile is j=8h+7, i.e. t=2h+1), so
                            # only h=NH-1's store trails the final matmul.
                            if t % 2 == 1:
                                h = (t - 1) // 2
                                # Unscaled copy frees the PSUM bank immediately
                                # so the next cs's accumulation never waits on
                                # the reciprocal (which waits on the whole
                                # score phase).
                                stg = avstage.tile(
                                    [128, 512], f32, tag="avs", name=f"avs{h}_{cs}"
                                )
                                nc.vector.tensor_copy(out=stg[:], in_=av[h][:])
                                ob = outp.tile([128, 512], f32, tag="out")
                                nc.scalar.activation(
                                    out=ob[:], in_=stg[:], func=CPY,
                                    scale=recip_sb[:, h:h + 1],
                                )
                                nc.scalar.dma_start(
                                    out=out[128 * h:128 * (h + 1), 512 * cs:512 * (cs + 1)],
                                    in_=ob[:],
                                )

    nc.finalize()
    return nc


def _prep_inputs(x, Wq, Wk, Wv, S):
    import ml_dtypes

    bf = ml_dtypes.bfloat16
    ML = S // N_CORES

    def shuf_w(W):
        # [dt, p, c, j] layout: element = W[128c+p, 128dt+j]
        return np.ascontiguousarray(
            W.reshape(CC, 128, DT, 128).transpose(2, 1, 0, 3)
        ).astype(bf)

    wqk_h = shuf_w((Wq @ Wk.T).astype(np.float32))
    wv_h = np.ascontiguousarray(
        Wv.reshape(CC, 128, D).transpose(1, 0, 2)
    ).astype(bf)

    def shuf_x(rows):
        # rows [n, D] -> [p, c, m] with element = rows[m, 128c+p]
        n = rows.shape[0]
        return np.ascontiguousarray(rows.reshape(n, CC, 128).transpose(2, 1, 0)).astype(bf)

    xt_h = shuf_x(x)
    in_maps = []
    for i in range(N_CORES):
        mask = (np.arange(128)[:, None] <= 8 * np.arange(16)[None, :] + i).astype(bf)
        in_maps.append({
            "xq": shuf_x(x[i::N_CORES]),
            "xkv": shuf_x(x[ML * i:ML * (i + 1)]),
            "xt": xt_h,
            "wqk": wqk_h, "wv": wv_h,
            "mask": mask,
        })
    return in_maps


def run(x, Wq, Wk, Wv, S, trace=False, trace_cores=None):
    from concourse.bass_utils import run_bass_kernel_spmd

    if S not in _BUILT:
        _BUILT[S] = _build(S)
    nc = _BUILT[S]
    in_maps = _prep_inputs(x, Wq, Wk, Wv, S)
    res = run_bass_kernel_spmd(
        nc, in_maps, list(range(N_CORES)), trace=trace, trace_cores=trace_cores
    )
    outs = [res.results[i]["out"] for i in range(N_CORES)]
    full = np.stack(outs, axis=1).reshape(S, D).astype(np.float32)
    return full, res


def kernel(x, Wq, Wk, Wv):
    x = np.asarray(x, dtype=np.float32)
    Wq = np.asarray(Wq, dtype=np.float32)
    Wk = np.asarray(Wk, dtype=np.float32)
    Wv = np.asarray(Wv, dtype=np.float32)
    full, _ = run(x, Wq, Wk, Wv, x.shape[0])
    return full
